# revision 8
# baseline (speedup 1.0000x reference)
"""Multi-Head Latent Attention (MLA) prefill kernel for 8x Trainium2 NeuronCores.

Reference computes:
    compressed_q  = LN(x @ W_dq)            [B,L,512]
    Q             = compressed_q @ W_uq     [B,L,1024]
    compressed_kv = LN(x @ W_dkv)           [B,L,682]
    K             = (compressed_kv @ W_ukv)[..., :1024]
    attn          = softmax(q @ k^T / 8)    [B,16,L,L]
    returns (x, attn, compressed_kv)        (V path is dead code)

Sharding: 8 cores = 2 batches x 4 head-groups (4 heads each).  Each core
computes the full down-proj + LN for its batch (replicated within the
batch group) and the up-projections/attention for its 4 heads only.
Host-side prep (free): x is passed pre-transposed, up-proj weights are
pre-sliced per head group and cast to fp16, W_uk is zero-padded 682->768.

All matmuls run in fp16 (single-pass on the PE; fp32 is 2-pass): products
are ~0.01 scale, comfortably inside fp16 range, and the fp32 PSUM
accumulate keeps dot-product error ~1e-4 relative.  LayerNorm stats,
softmax and all outputs stay fp32.  Latent transposes use the DMA xbar
(2-byte dtype) instead of the tensor engine.
"""

import numpy as np

import concourse.bass as bass
import concourse.tile as tile
from concourse import bacc, mybir
from concourse.bass_utils import run_bass_kernel_spmd
from concourse.masks import make_identity

F32 = mybir.dt.float32
F16 = mybir.dt.float16
AF = mybir.ActivationFunctionType
ALU = mybir.AluOpType

D_MODEL = 1024
N_HEADS = 16
DH = 64
Q_PROJ = 512
KV_PROJ = 682
KV_PAD = 768          # 682 padded to 6*128 for partition tiling
LN_EPS = 1e-5
B = 2
L_FULL = 2048
HPC = 4               # heads per core
N_CORES = 8

TRACE = False
TRACE_ALL_CORES = False
LAST_RESULT = None
_NC_CACHE = {}


def build(L=L_FULL, ln_affine=False):
    IT = L // 128          # i-tiles (query rows)
    JC = max(1, L // 512)  # 512-wide column chunks
    KB1 = 512 if KV_PROJ > 512 else KV_PROJ

    nc = bacc.Bacc(None, target_bir_lowering=False)

    def rstd_newton(pool, var_ap, tag):
        """rstd = 1/sqrt(var+eps), Sqrt-spline + one Newton step (fp32-exact)."""
        ve = pool.tile([128, 1], F32, name=f"ve_{tag}", tag=f"ve_{tag}")
        nc.vector.tensor_scalar_add(ve[:], var_ap, LN_EPS)
        rt = pool.tile([128, 1], F32, name=f"rt_{tag}", tag=f"rt_{tag}")
        nc.scalar.activation(rt[:], ve[:], AF.Sqrt)
        nc.vector.reciprocal(rt[:], rt[:])
        rr = pool.tile([128, 1], F32, name=f"rr_{tag}", tag=f"rr_{tag}")
        nc.vector.tensor_mul(rr[:], rt[:], rt[:])
        nc.vector.tensor_mul(rr[:], rr[:], ve[:])
        nc.vector.tensor_scalar(out=rr[:], in0=rr[:], scalar1=-0.5, scalar2=1.5,
                                op0=ALU.mult, op1=ALU.add)
        nc.vector.tensor_mul(rt[:], rt[:], rr[:])
        return rt

    xT = nc.dram_tensor("xT", [D_MODEL, L], F32, kind="ExternalInput")
    wdq = nc.dram_tensor("wdq", [D_MODEL, Q_PROJ], F16, kind="ExternalInput")
    wdkv = nc.dram_tensor("wdkv", [D_MODEL, KV_PROJ], F16, kind="ExternalInput")
    wuq = nc.dram_tensor("wuq", [Q_PROJ, HPC * DH], F16, kind="ExternalInput")
    wuk = nc.dram_tensor("wuk", [KV_PAD, HPC * DH], F16, kind="ExternalInput")
    attn = nc.dram_tensor("attn", [HPC, L, L], F32, kind="ExternalOutput")
    ckv_o = nc.dram_tensor("ckv", [L, KV_PROJ], F32, kind="ExternalOutput")
    gbin = {}
    if ln_affine:
        for nm, n in (("qg", Q_PROJ), ("qb", Q_PROJ),
                      ("kg", KV_PROJ), ("kb", KV_PROJ)):
            gbin[nm] = nc.dram_tensor(nm, [1, n], F32, kind="ExternalInput")

    with tile.TileContext(nc) as tc:
        with tc.tile_pool(name="persist", bufs=1) as persist, \
             tc.tile_pool(name="latent", bufs=1) as latent:
            ident = persist.tile([128, 128], F16)
            make_identity(nc, ident[:])
            wuq_sb = persist.tile([128, 4, HPC * DH], F16)
            nc.gpsimd.dma_start(out=wuq_sb[:],
                              in_=wuq[:, :].rearrange("(t p) m -> p t m", p=128))
            wuk_sb = persist.tile([128, 6, HPC * DH], F16)
            nc.gpsimd.dma_start(out=wuk_sb[:],
                              in_=wuk[:, :].rearrange("(t p) m -> p t m", p=128))
            # replicated gamma/beta rows (only when LN is affine-nontrivial)
            gbt = {}
            if ln_affine:
                for nm, n in (("qg", Q_PROJ), ("qb", Q_PROJ),
                              ("kg", KV_PROJ), ("kb", KV_PROJ)):
                    t = persist.tile([128, n], F32, name=f"gb_{nm}", tag=f"gb_{nm}")
                    src = gbin[nm][:, :]
                    rep = bass.AP(tensor=src.tensor, offset=src.offset,
                                  ap=[[0, 128], src.ap[1]])
                    nc.sync.dma_start(out=t[:], in_=rep)
                    gbt[nm] = t
            qT = [persist.tile([128, L], F16, name=f"qT{i}", tag=f"qT{i}") for i in range(2)]
            kT = [persist.tile([128, L], F16, name=f"kT{i}", tag=f"kT{i}") for i in range(2)]
            cqT = [latent.tile([128, L], F16, name=f"cqT{i}", tag=f"cqT{i}") for i in range(4)]
            ckvT = [latent.tile([128, L], F16, name=f"ckvT{i}", tag=f"ckvT{i}") for i in range(6)]

            # ---------------- phase 1+2: down-proj, LN, ckv out, transposes
            with tc.tile_pool(name="wdown", bufs=1) as wdown, \
                 tc.tile_pool(name="ph1", bufs=2) as ph1, \
                 tc.tile_pool(name="ps1", bufs=2, space="PSUM") as ps1, \
                 tc.tile_pool(name="pst", bufs=2, space="PSUM") as pst:
                wdq_sb = wdown.tile([128, 8, Q_PROJ], F16)
                nc.gpsimd.dma_start(out=wdq_sb[:],
                                  in_=wdq[:, :].rearrange("(k p) c -> p k c", p=128))
                wdkv_sb = wdown.tile([128, 8, KV_PROJ], F16)
                nc.gpsimd.dma_start(out=wdkv_sb[:],
                                  in_=wdkv[:, :].rearrange("(k p) c -> p k c", p=128))
                xTr = xT[:, :].rearrange("(k p) i -> p k i", p=128)
                for it in range(IT):
                    s = it * 128
                    with nc.named_scope("down"):
                        # SWDGE cast-DMA: fp32 HBM -> fp16 SBUF
                        xt = ph1.tile([128, 8, 128], F16, tag="xt")
                        nc.gpsimd.dma_start(out=xt[:], in_=xTr[:, :, s:s + 128])
                        psq = ps1.tile([128, Q_PROJ], F32, tag="psq")
                        pskv = ps1.tile([128, KV_PROJ], F32, tag="pskv")
                        for k in range(8):
                            st, sp = (k == 0), (k == 7)
                            nc.tensor.matmul(psq[:], xt[:, k, :], wdq_sb[:, k, :],
                                             start=st, stop=sp)
                            nc.tensor.matmul(pskv[:, 0:KB1], xt[:, k, :],
                                             wdkv_sb[:, k, 0:KB1], start=st, stop=sp)
                            nc.tensor.matmul(pskv[:, KB1:KV_PROJ], xt[:, k, :],
                                             wdkv_sb[:, k, KB1:KV_PROJ],
                                             start=st, stop=sp)
                    with nc.named_scope("ln"):
                        # --- LN(q) -> fp16 latent
                        stq = ph1.tile([128, 6], F32, tag="stq")
                        nc.vector.bn_stats(stq[:], psq[:])
                        mvq = ph1.tile([128, 2], F32, tag="mvq")
                        nc.vector.bn_aggr(mvq[:], stq[:])
                        rtq = rstd_newton(ph1, mvq[:, 1:2], "q")
                        if ln_affine:
                            cqf = ph1.tile([128, Q_PROJ], F32, tag="cqf")
                            nc.vector.tensor_scalar(out=cqf[:], in0=psq[:],
                                                    scalar1=mvq[:, 0:1], scalar2=rtq[:],
                                                    op0=ALU.subtract, op1=ALU.mult)
                            nc.vector.tensor_mul(cqf[:], cqf[:], gbt["qg"][:])
                            nc.vector.tensor_add(cqf[:], cqf[:], gbt["qb"][:])
                            cqh = ph1.tile([128, Q_PROJ], F16, tag="cqh")
                            nc.vector.tensor_copy(cqh[:], cqf[:])
                        else:
                            cqh = ph1.tile([128, Q_PROJ], F16, tag="cqh")
                            nc.vector.tensor_scalar(out=cqh[:], in0=psq[:],
                                                    scalar1=mvq[:, 0:1], scalar2=rtq[:],
                                                    op0=ALU.subtract, op1=ALU.mult)
                        # --- LN(kv): fp32 output + fp16 latent
                        stkv = ph1.tile([128, 2, 6], F32, tag="stkv")
                        nc.vector.bn_stats(stkv[:, 0, :], pskv[:, 0:341])
                        nc.vector.bn_stats(stkv[:, 1, :], pskv[:, 341:682])
                        mvk = ph1.tile([128, 2], F32, tag="mvk")
                        nc.vector.bn_aggr(mvk[:], stkv[:])
                        rtk = rstd_newton(ph1, mvk[:, 1:2], "k")
                        ckv = ph1.tile([128, KV_PROJ], F32, tag="ckv")
                        nc.vector.tensor_scalar(out=ckv[:], in0=pskv[:],
                                                scalar1=mvk[:, 0:1], scalar2=rtk[:],
                                                op0=ALU.subtract, op1=ALU.mult)
                        if ln_affine:
                            nc.vector.tensor_mul(ckv[:], ckv[:], gbt["kg"][:])
                            nc.vector.tensor_add(ckv[:], ckv[:], gbt["kb"][:])
                        nc.gpsimd.dma_start(out=ckv_o[s:s + 128, :], in_=ckv[:])
                        ckvh = ph1.tile([128, KV_PAD], F16, tag="ckvh")
                        nc.vector.tensor_copy(ckvh[:, 0:KV_PROJ], ckv[:])
                        nc.vector.memset(ckvh[:, KV_PROJ:KV_PAD], 0.0)
                    with nc.named_scope("tpose"):
                        # PE transposes (fp16), evacuated by DVE/ACT copies
                        for ct in range(4):
                            pt = pst.tile([128, 128], F16, tag="pt")
                            nc.tensor.transpose(pt[:], cqh[:, ct * 128:(ct + 1) * 128],
                                                ident[:])
                            nc.any.tensor_copy(cqT[ct][:, s:s + 128], pt[:])
                        for ct in range(6):
                            pt = pst.tile([128, 128], F16, tag="pt")
                            nc.tensor.transpose(pt[:], ckvh[:, ct * 128:(ct + 1) * 128],
                                                ident[:])
                            nc.any.tensor_copy(ckvT[ct][:, s:s + 128], pt[:])

            # ---------------- phase 3: up-projections -> qT, kT (d-major, fp16)
            with tc.tile_pool(name="ps3", bufs=4, space="PSUM") as ps3, \
                 nc.named_scope("upproj"):
                for mt in range(2):
                    for ncn in range(JC):
                        t = ncn * 512
                        w = min(512, L - t)
                        pu = ps3.tile([128, 512], F32, tag="pu")
                        for ct in range(4):
                            nc.tensor.matmul(pu[:, :w],
                                             wuq_sb[:, ct, mt * 128:(mt + 1) * 128],
                                             cqT[ct][:, t:t + w],
                                             start=(ct == 0), stop=(ct == 3))
                        nc.any.tensor_copy(qT[mt][:, t:t + w], pu[:, :w])
                        pk = ps3.tile([128, 512], F32, tag="pk")
                        for ct in range(6):
                            nc.tensor.matmul(pk[:, :w],
                                             wuk_sb[:, ct, mt * 128:(mt + 1) * 128],
                                             ckvT[ct][:, t:t + w],
                                             start=(ct == 0), stop=(ct == 5))
                        nc.any.tensor_copy(kT[mt][:, t:t + w], pk[:, :w])

            # ---------------- phase 4: scores, softmax, output
            with tc.tile_pool(name="ph4", bufs=3) as ph4, \
                 tc.tile_pool(name="st4", bufs=4) as st4, \
                 tc.tile_pool(name="ps4", bufs=2, space="PSUM") as ps4, \
                 nc.named_scope("attn"):
                for h in range(HPC):
                    mt, po = h // 2, (h % 2) * 64
                    for it in range(IT):
                        s = it * 128
                        ps = ps4.tile([128, L], F32, tag="ps")
                        for jc in range(JC):
                            t = jc * 512
                            nc.tensor.matmul(ps[:, t:t + 512],
                                             qT[mt][po:po + 64, s:s + 128],
                                             kT[mt][po:po + 64, t:t + 512],
                                             start=True, stop=True)
                        ex = ph4.tile([128, L], F32, tag="ex")
                        tot = st4.tile([128, 1], F32, tag="tot")
                        nc.scalar.activation(ex[:], ps[:], AF.Exp, scale=0.125,
                                             accum_out=tot[:])
                        nc.vector.reciprocal(tot[:], tot[:])
                        nc.gpsimd.tensor_scalar_mul(out=ex[:], in0=ex[:],
                                                    scalar1=tot[:])
                        nc.sync.dma_start(out=attn[h, s:s + 128, :], in_=ex[:])

    nc.compile()
    return nc


def _get_nc(L, ln_affine):
    key = (L, ln_affine)
    if key not in _NC_CACHE:
        _NC_CACHE[key] = build(L, ln_affine)
    return _NC_CACHE[key]


def _prep_in_maps(x, W_dq, W_uq, q_gamma, q_beta, W_dkv, W_ukv, kv_gamma, kv_beta,
                  ln_affine):
    f32 = lambda a: np.ascontiguousarray(np.asarray(a), dtype=np.float32)
    f16 = lambda a: np.ascontiguousarray(np.asarray(a, dtype=np.float32)).astype(np.float16)
    x = f32(x)
    xTs = [np.ascontiguousarray(x[b].T) for b in range(x.shape[0])]
    W_uq = np.asarray(W_uq, dtype=np.float32)
    W_ukv = np.asarray(W_ukv, dtype=np.float32)
    wuk_full = np.zeros((KV_PAD, D_MODEL), np.float32)
    wuk_full[:KV_PROJ] = W_ukv[:, :D_MODEL]   # K half only; V half is dead
    wdq16, wdkv16 = f16(W_dq), f16(W_dkv)
    in_maps = []
    for c in range(N_CORES):
        b, hg = c // HPC, c % HPC
        sl = slice(hg * HPC * DH, (hg + 1) * HPC * DH)
        m = {
            "xT": xTs[b],
            "wdq": wdq16,
            "wdkv": wdkv16,
            "wuq": f16(W_uq[:, sl]),
            "wuk": f16(wuk_full[:, sl]),
        }
        if ln_affine:
            m["qg"] = f32(q_gamma).reshape(1, Q_PROJ)
            m["qb"] = f32(q_beta).reshape(1, Q_PROJ)
            m["kg"] = f32(kv_gamma).reshape(1, KV_PROJ)
            m["kb"] = f32(kv_beta).reshape(1, KV_PROJ)
        in_maps.append(m)
    return x, in_maps


def kernel(x, W_dq, W_uq, q_gamma, q_beta, W_dkv, W_ukv, kv_gamma, kv_beta):
    global LAST_RESULT
    qg, qb = np.asarray(q_gamma), np.asarray(q_beta)
    kg, kb = np.asarray(kv_gamma), np.asarray(kv_beta)
    ln_affine = not (np.all(qg == 1.0) and np.all(qb == 0.0)
                     and np.all(kg == 1.0) and np.all(kb == 0.0))
    x, in_maps = _prep_in_maps(x, W_dq, W_uq, q_gamma, q_beta,
                               W_dkv, W_ukv, kv_gamma, kv_beta, ln_affine)
    Bx, L, _ = x.shape
    nc = _get_nc(L, ln_affine)
    kw = {}
    if TRACE:
        kw["trace"] = True
        if TRACE_ALL_CORES:
            kw["trace_cores"] = list(range(N_CORES))
            kw["stitch_traces"] = True
    res = run_bass_kernel_spmd(nc, in_maps, core_ids=list(range(N_CORES)), **kw)
    LAST_RESULT = res
    attn = np.empty((Bx, N_HEADS, L, L), np.float32)
    for c in range(N_CORES):
        b, hg = c // HPC, c % HPC
        attn[b, hg * HPC:(hg + 1) * HPC] = res.results[c]["attn"]
    ckv = np.stack([res.results[0]["ckv"], res.results[HPC]["ckv"]])
    return (x, attn, ckv)


# revision 9
# speedup vs baseline: 5.6086x; 5.6086x over previous
"""Multi-Head Latent Attention (MLA) prefill kernel for 8x Trainium2 NeuronCores.

Reference computes:
    compressed_q  = LN(x @ W_dq)            [B,L,512]
    Q             = compressed_q @ W_uq     [B,L,1024]
    compressed_kv = LN(x @ W_dkv)           [B,L,682]
    K             = (compressed_kv @ W_ukv)[..., :1024]
    attn          = softmax(q @ k^T / 8)    [B,16,L,L]
    returns (x, attn, compressed_kv)        (V path is dead code)

Sharding: 8 cores = 2 batches x 4 head-groups (4 heads each).  Each core
computes the full down-proj + LN for its batch (replicated within the
batch group) and the up-projections/attention for its 4 heads only.
Host-side prep (free): x is passed pre-transposed, up-proj weights are
pre-sliced per head group and cast to fp16, W_uk is zero-padded 682->768.

All matmuls run in fp16 (single-pass on the PE; fp32 is 2-pass): products
are ~0.01 scale, comfortably inside fp16 range, and the fp32 PSUM
accumulate keeps dot-product error ~1e-4 relative.  LayerNorm stats,
softmax and all outputs stay fp32.  Latent transposes use the DMA xbar
(2-byte dtype) instead of the tensor engine.
"""

import numpy as np

import concourse.bass as bass
import concourse.tile as tile
from concourse import bacc, mybir
from concourse.bass_utils import run_bass_kernel_spmd
from concourse.masks import make_identity

F32 = mybir.dt.float32
F16 = mybir.dt.float16
AF = mybir.ActivationFunctionType
ALU = mybir.AluOpType

D_MODEL = 1024
N_HEADS = 16
DH = 64
Q_PROJ = 512
KV_PROJ = 682
KV_PAD = 768          # 682 padded to 6*128 for partition tiling
LN_EPS = 1e-5
B = 2
L_FULL = 2048
HPC = 4               # heads per core
N_CORES = 8

TRACE = False
TRACE_ALL_CORES = False
LAST_RESULT = None
_NC_CACHE = {}


def build(L=L_FULL, ln_affine=False):
    IT = L // 128          # i-tiles (query rows)
    JC = max(1, L // 512)  # 512-wide column chunks
    KB1 = 512 if KV_PROJ > 512 else KV_PROJ

    nc = bacc.Bacc(None, target_bir_lowering=False)

    def rstd_newton(pool, var_ap, tag):
        """rstd = 1/sqrt(var+eps), Sqrt-spline + one Newton step (fp32-exact)."""
        ve = pool.tile([128, 1], F32, name=f"ve_{tag}", tag=f"ve_{tag}")
        nc.vector.tensor_scalar_add(ve[:], var_ap, LN_EPS)
        rt = pool.tile([128, 1], F32, name=f"rt_{tag}", tag=f"rt_{tag}")
        nc.scalar.activation(rt[:], ve[:], AF.Sqrt)
        nc.vector.reciprocal(rt[:], rt[:])
        rr = pool.tile([128, 1], F32, name=f"rr_{tag}", tag=f"rr_{tag}")
        nc.vector.tensor_mul(rr[:], rt[:], rt[:])
        nc.vector.tensor_mul(rr[:], rr[:], ve[:])
        nc.vector.tensor_scalar(out=rr[:], in0=rr[:], scalar1=-0.5, scalar2=1.5,
                                op0=ALU.mult, op1=ALU.add)
        nc.vector.tensor_mul(rt[:], rt[:], rr[:])
        return rt

    xT = nc.dram_tensor("xT", [D_MODEL, L], F32, kind="ExternalInput")
    wdq = nc.dram_tensor("wdq", [D_MODEL, Q_PROJ], F16, kind="ExternalInput")
    wdkv = nc.dram_tensor("wdkv", [D_MODEL, KV_PROJ], F16, kind="ExternalInput")
    wuq = nc.dram_tensor("wuq", [Q_PROJ, HPC * DH], F16, kind="ExternalInput")
    wuk = nc.dram_tensor("wuk", [KV_PAD, HPC * DH], F16, kind="ExternalInput")
    attn = nc.dram_tensor("attn", [HPC, L, L], F32, kind="ExternalOutput")
    ckv_o = nc.dram_tensor("ckv", [L, KV_PROJ], F32, kind="ExternalOutput")
    gbin = {}
    if ln_affine:
        for nm, n in (("qg", Q_PROJ), ("qb", Q_PROJ),
                      ("kg", KV_PROJ), ("kb", KV_PROJ)):
            gbin[nm] = nc.dram_tensor(nm, [1, n], F32, kind="ExternalInput")

    with tile.TileContext(nc) as tc:
        with tc.tile_pool(name="persist", bufs=1) as persist, \
             tc.tile_pool(name="latent", bufs=1) as latent:
            ident = persist.tile([128, 128], F16)
            make_identity(nc, ident[:])
            wuq_sb = persist.tile([128, 4, HPC * DH], F16)
            nc.gpsimd.dma_start(out=wuq_sb[:],
                              in_=wuq[:, :].rearrange("(t p) m -> p t m", p=128))
            wuk_sb = persist.tile([128, 6, HPC * DH], F16)
            nc.gpsimd.dma_start(out=wuk_sb[:],
                              in_=wuk[:, :].rearrange("(t p) m -> p t m", p=128))
            # replicated gamma/beta rows (only when LN is affine-nontrivial)
            gbt = {}
            if ln_affine:
                for nm, n in (("qg", Q_PROJ), ("qb", Q_PROJ),
                              ("kg", KV_PROJ), ("kb", KV_PROJ)):
                    t = persist.tile([128, n], F32, name=f"gb_{nm}", tag=f"gb_{nm}")
                    src = gbin[nm][:, :]
                    rep = bass.AP(tensor=src.tensor, offset=src.offset,
                                  ap=[[0, 128], src.ap[1]])
                    nc.sync.dma_start(out=t[:], in_=rep)
                    gbt[nm] = t
            qT = [persist.tile([128, L], F16, name=f"qT{i}", tag=f"qT{i}") for i in range(2)]
            kT = [persist.tile([128, L], F16, name=f"kT{i}", tag=f"kT{i}") for i in range(2)]
            cqT = [latent.tile([128, L], F16, name=f"cqT{i}", tag=f"cqT{i}") for i in range(4)]
            ckvT = [latent.tile([128, L], F16, name=f"ckvT{i}", tag=f"ckvT{i}") for i in range(6)]

            # ---------------- phase 1+2: down-proj, LN, ckv out, transposes
            with tc.tile_pool(name="wdown", bufs=1) as wdown, \
                 tc.tile_pool(name="ph1", bufs=2) as ph1, \
                 tc.tile_pool(name="ps1", bufs=2, space="PSUM") as ps1, \
                 tc.tile_pool(name="pst", bufs=2, space="PSUM") as pst:
                wdq_sb = wdown.tile([128, 8, Q_PROJ], F16)
                nc.gpsimd.dma_start(out=wdq_sb[:],
                                  in_=wdq[:, :].rearrange("(k p) c -> p k c", p=128))
                wdkv_sb = wdown.tile([128, 8, KV_PROJ], F16)
                nc.gpsimd.dma_start(out=wdkv_sb[:],
                                  in_=wdkv[:, :].rearrange("(k p) c -> p k c", p=128))
                xTr = xT[:, :].rearrange("(k p) i -> p k i", p=128)
                for it in range(IT):
                    s = it * 128
                    with nc.named_scope("down"):
                        # SWDGE cast-DMA: fp32 HBM -> fp16 SBUF
                        xt = ph1.tile([128, 8, 128], F16, tag="xt")
                        nc.gpsimd.dma_start(out=xt[:], in_=xTr[:, :, s:s + 128])
                        psq = ps1.tile([128, Q_PROJ], F32, tag="psq")
                        pskv = ps1.tile([128, KV_PROJ], F32, tag="pskv")
                        for k in range(8):
                            st, sp = (k == 0), (k == 7)
                            nc.tensor.matmul(psq[:], xt[:, k, :], wdq_sb[:, k, :],
                                             start=st, stop=sp)
                            nc.tensor.matmul(pskv[:, 0:KB1], xt[:, k, :],
                                             wdkv_sb[:, k, 0:KB1], start=st, stop=sp)
                            nc.tensor.matmul(pskv[:, KB1:KV_PROJ], xt[:, k, :],
                                             wdkv_sb[:, k, KB1:KV_PROJ],
                                             start=st, stop=sp)
                    with nc.named_scope("ln"):
                        # --- LN(q) -> fp16 latent
                        stq = ph1.tile([128, 6], F32, tag="stq")
                        nc.vector.bn_stats(stq[:], psq[:])
                        mvq = ph1.tile([128, 2], F32, tag="mvq")
                        nc.vector.bn_aggr(mvq[:], stq[:])
                        rtq = rstd_newton(ph1, mvq[:, 1:2], "q")
                        if ln_affine:
                            cqf = ph1.tile([128, Q_PROJ], F32, tag="cqf")
                            nc.vector.tensor_scalar(out=cqf[:], in0=psq[:],
                                                    scalar1=mvq[:, 0:1], scalar2=rtq[:],
                                                    op0=ALU.subtract, op1=ALU.mult)
                            nc.vector.tensor_mul(cqf[:], cqf[:], gbt["qg"][:])
                            nc.vector.tensor_add(cqf[:], cqf[:], gbt["qb"][:])
                            cqh = ph1.tile([128, Q_PROJ], F16, tag="cqh")
                            nc.vector.tensor_copy(cqh[:], cqf[:])
                        else:
                            cqh = ph1.tile([128, Q_PROJ], F16, tag="cqh")
                            nc.vector.tensor_scalar(out=cqh[:], in0=psq[:],
                                                    scalar1=mvq[:, 0:1], scalar2=rtq[:],
                                                    op0=ALU.subtract, op1=ALU.mult)
                        # --- LN(kv): fp32 output + fp16 latent
                        stkv = ph1.tile([128, 2, 6], F32, tag="stkv")
                        nc.vector.bn_stats(stkv[:, 0, :], pskv[:, 0:341])
                        nc.vector.bn_stats(stkv[:, 1, :], pskv[:, 341:682])
                        mvk = ph1.tile([128, 2], F32, tag="mvk")
                        nc.vector.bn_aggr(mvk[:], stkv[:])
                        rtk = rstd_newton(ph1, mvk[:, 1:2], "k")
                        ckv = ph1.tile([128, KV_PROJ], F32, tag="ckv")
                        nc.vector.tensor_scalar(out=ckv[:], in0=pskv[:],
                                                scalar1=mvk[:, 0:1], scalar2=rtk[:],
                                                op0=ALU.subtract, op1=ALU.mult)
                        if ln_affine:
                            nc.vector.tensor_mul(ckv[:], ckv[:], gbt["kg"][:])
                            nc.vector.tensor_add(ckv[:], ckv[:], gbt["kb"][:])
                        nc.gpsimd.dma_start(out=ckv_o[s:s + 128, :], in_=ckv[:])
                        ckvh = ph1.tile([128, KV_PAD], F16, tag="ckvh")
                        nc.vector.tensor_copy(ckvh[:, 0:KV_PROJ], ckv[:])
                        nc.vector.memset(ckvh[:, KV_PROJ:KV_PAD], 0.0)
                    with nc.named_scope("tpose"):
                        # PE transposes (fp16), evacuated by DVE/ACT copies
                        for ct in range(4):
                            pt = pst.tile([128, 128], F16, tag="pt")
                            nc.tensor.transpose(pt[:], cqh[:, ct * 128:(ct + 1) * 128],
                                                ident[:])
                            nc.any.tensor_copy(cqT[ct][:, s:s + 128], pt[:])
                        for ct in range(6):
                            pt = pst.tile([128, 128], F16, tag="pt")
                            nc.tensor.transpose(pt[:], ckvh[:, ct * 128:(ct + 1) * 128],
                                                ident[:])
                            nc.any.tensor_copy(ckvT[ct][:, s:s + 128], pt[:])

            # ---------------- phase 3: up-projections -> qT, kT (d-major, fp16)
            with tc.tile_pool(name="ps3", bufs=4, space="PSUM") as ps3, \
                 nc.named_scope("upproj"):
                for mt in range(2):
                    for ncn in range(JC):
                        t = ncn * 512
                        w = min(512, L - t)
                        pu = ps3.tile([128, 512], F32, tag="pu")
                        for ct in range(4):
                            nc.tensor.matmul(pu[:, :w],
                                             wuq_sb[:, ct, mt * 128:(mt + 1) * 128],
                                             cqT[ct][:, t:t + w],
                                             start=(ct == 0), stop=(ct == 3))
                        nc.any.tensor_copy(qT[mt][:, t:t + w], pu[:, :w])
                        pk = ps3.tile([128, 512], F32, tag="pk")
                        for ct in range(6):
                            nc.tensor.matmul(pk[:, :w],
                                             wuk_sb[:, ct, mt * 128:(mt + 1) * 128],
                                             ckvT[ct][:, t:t + w],
                                             start=(ct == 0), stop=(ct == 5))
                        nc.any.tensor_copy(kT[mt][:, t:t + w], pk[:, :w])

            # ---------------- phase 4: scores, softmax, output
            with tc.tile_pool(name="ph4", bufs=3) as ph4, \
                 tc.tile_pool(name="st4", bufs=4) as st4, \
                 tc.tile_pool(name="ps4", bufs=2, space="PSUM") as ps4, \
                 nc.named_scope("attn"):
                for h in range(HPC):
                    mt, po = h // 2, (h % 2) * 64
                    for it in range(IT):
                        s = it * 128
                        ps = ps4.tile([128, L], F32, tag="ps")
                        for jc in range(JC):
                            t = jc * 512
                            nc.tensor.matmul(ps[:, t:t + 512],
                                             qT[mt][po:po + 64, s:s + 128],
                                             kT[mt][po:po + 64, t:t + 512],
                                             start=True, stop=True)
                        ex = ph4.tile([128, L], F32, tag="ex")
                        tot = st4.tile([128, 1], F32, tag="tot")
                        nc.scalar.activation(ex[:], ps[:], AF.Exp, scale=0.125,
                                             accum_out=tot[:])
                        nc.vector.reciprocal(tot[:], tot[:])
                        nc.vector.tensor_scalar_mul(out=ex[:], in0=ex[:],
                                                    scalar1=tot[:])
                        nc.sync.dma_start(out=attn[h, s:s + 128, :], in_=ex[:])

    nc.compile()
    return nc


def _get_nc(L, ln_affine):
    key = (L, ln_affine)
    if key not in _NC_CACHE:
        _NC_CACHE[key] = build(L, ln_affine)
    return _NC_CACHE[key]


def _prep_in_maps(x, W_dq, W_uq, q_gamma, q_beta, W_dkv, W_ukv, kv_gamma, kv_beta,
                  ln_affine):
    f32 = lambda a: np.ascontiguousarray(np.asarray(a), dtype=np.float32)
    f16 = lambda a: np.ascontiguousarray(np.asarray(a, dtype=np.float32)).astype(np.float16)
    x = f32(x)
    xTs = [np.ascontiguousarray(x[b].T) for b in range(x.shape[0])]
    W_uq = np.asarray(W_uq, dtype=np.float32)
    W_ukv = np.asarray(W_ukv, dtype=np.float32)
    wuk_full = np.zeros((KV_PAD, D_MODEL), np.float32)
    wuk_full[:KV_PROJ] = W_ukv[:, :D_MODEL]   # K half only; V half is dead
    wdq16, wdkv16 = f16(W_dq), f16(W_dkv)
    in_maps = []
    for c in range(N_CORES):
        b, hg = c // HPC, c % HPC
        sl = slice(hg * HPC * DH, (hg + 1) * HPC * DH)
        m = {
            "xT": xTs[b],
            "wdq": wdq16,
            "wdkv": wdkv16,
            "wuq": f16(W_uq[:, sl]),
            "wuk": f16(wuk_full[:, sl]),
        }
        if ln_affine:
            m["qg"] = f32(q_gamma).reshape(1, Q_PROJ)
            m["qb"] = f32(q_beta).reshape(1, Q_PROJ)
            m["kg"] = f32(kv_gamma).reshape(1, KV_PROJ)
            m["kb"] = f32(kv_beta).reshape(1, KV_PROJ)
        in_maps.append(m)
    return x, in_maps


def kernel(x, W_dq, W_uq, q_gamma, q_beta, W_dkv, W_ukv, kv_gamma, kv_beta):
    global LAST_RESULT
    qg, qb = np.asarray(q_gamma), np.asarray(q_beta)
    kg, kb = np.asarray(kv_gamma), np.asarray(kv_beta)
    ln_affine = not (np.all(qg == 1.0) and np.all(qb == 0.0)
                     and np.all(kg == 1.0) and np.all(kb == 0.0))
    x, in_maps = _prep_in_maps(x, W_dq, W_uq, q_gamma, q_beta,
                               W_dkv, W_ukv, kv_gamma, kv_beta, ln_affine)
    Bx, L, _ = x.shape
    nc = _get_nc(L, ln_affine)
    kw = {}
    if TRACE:
        kw["trace"] = True
        if TRACE_ALL_CORES:
            kw["trace_cores"] = list(range(N_CORES))
            kw["stitch_traces"] = True
    res = run_bass_kernel_spmd(nc, in_maps, core_ids=list(range(N_CORES)), **kw)
    LAST_RESULT = res
    attn = np.empty((Bx, N_HEADS, L, L), np.float32)
    for c in range(N_CORES):
        b, hg = c // HPC, c % HPC
        attn[b, hg * HPC:(hg + 1) * HPC] = res.results[c]["attn"]
    ckv = np.stack([res.results[0]["ckv"], res.results[HPC]["ckv"]])
    return (x, attn, ckv)


# revision 11
# speedup vs baseline: 6.0181x; 1.0730x over previous
"""Multi-Head Latent Attention (MLA) prefill kernel for 8x Trainium2 NeuronCores.

Reference computes:
    compressed_q  = LN(x @ W_dq)            [B,L,512]
    Q             = compressed_q @ W_uq     [B,L,1024]
    compressed_kv = LN(x @ W_dkv)           [B,L,682]
    K             = (compressed_kv @ W_ukv)[..., :1024]
    attn          = softmax(q @ k^T / 8)    [B,16,L,L]
    returns (x, attn, compressed_kv)        (V path is dead code)

Sharding: 8 cores = 2 batches x 4 head-groups (4 heads each).  Each core
computes the full down-proj + LN for its batch (replicated within the
batch group) and the up-projections/attention for its 4 heads only.
Host-side prep (free): x is passed pre-transposed, up-proj weights are
pre-sliced per head group and cast to fp16, W_uk is zero-padded 682->768.

All matmuls run in fp16 (single-pass on the PE; fp32 is 2-pass): products
are ~0.01 scale, comfortably inside fp16 range, and the fp32 PSUM
accumulate keeps dot-product error ~1e-4 relative.  LayerNorm stats,
softmax and all outputs stay fp32.  Latent transposes use the DMA xbar
(2-byte dtype) instead of the tensor engine.
"""

import numpy as np

import concourse.bass as bass
import concourse.tile as tile
from concourse import bacc, mybir
from concourse.bass_utils import run_bass_kernel_spmd
from concourse.masks import make_identity

F32 = mybir.dt.float32
F16 = mybir.dt.float16
AF = mybir.ActivationFunctionType
ALU = mybir.AluOpType

D_MODEL = 1024
N_HEADS = 16
DH = 64
Q_PROJ = 512
KV_PROJ = 682
KV_PAD = 768          # 682 padded to 6*128 for partition tiling
LN_EPS = 1e-5
B = 2
L_FULL = 2048
HPC = 4               # heads per core
N_CORES = 8

TRACE = False
TRACE_ALL_CORES = False
LAST_RESULT = None
_NC_CACHE = {}


def build(L=L_FULL, ln_affine=False):
    IT = L // 128          # i-tiles (query rows)
    JC = max(1, L // 512)  # 512-wide column chunks
    KB1 = 512 if KV_PROJ > 512 else KV_PROJ

    nc = bacc.Bacc(None, target_bir_lowering=False)

    epsv = None

    def rstd_newton(pool, var_ap, tag):
        """rstd = 1/sqrt(var+eps) via Sqrt spline + HW reciprocal."""
        rt = pool.tile([128, 1], F32, name=f"rt_{tag}", tag=f"rt_{tag}")
        nc.scalar.activation(rt[:], var_ap, AF.Sqrt, bias=epsv[:])
        nc.vector.reciprocal(rt[:], rt[:])
        return rt

    xT = nc.dram_tensor("xT", [D_MODEL, L], F32, kind="ExternalInput")
    wdq = nc.dram_tensor("wdq", [D_MODEL, Q_PROJ], F16, kind="ExternalInput")
    wdkv = nc.dram_tensor("wdkv", [D_MODEL, KV_PROJ], F16, kind="ExternalInput")
    wuq = nc.dram_tensor("wuq", [Q_PROJ, HPC * DH], F16, kind="ExternalInput")
    wuk = nc.dram_tensor("wuk", [KV_PAD, HPC * DH], F16, kind="ExternalInput")
    attn = nc.dram_tensor("attn", [HPC, L, L], F32, kind="ExternalOutput")
    ckv_o = nc.dram_tensor("ckv", [L, KV_PROJ], F32, kind="ExternalOutput")
    gbin = {}
    if ln_affine:
        for nm, n in (("qg", Q_PROJ), ("qb", Q_PROJ),
                      ("kg", KV_PROJ), ("kb", KV_PROJ)):
            gbin[nm] = nc.dram_tensor(nm, [1, n], F32, kind="ExternalInput")

    with tile.TileContext(nc) as tc:
        with tc.tile_pool(name="persist", bufs=1) as persist, \
             tc.tile_pool(name="latent", bufs=1) as latent:
            ident = persist.tile([128, 128], F16)
            make_identity(nc, ident[:])
            epsv = persist.tile([128, 1], F32)
            nc.vector.memset(epsv[:], LN_EPS)
            wuq_sb = persist.tile([128, 4, HPC * DH], F16)
            nc.gpsimd.dma_start(out=wuq_sb[:],
                              in_=wuq[:, :].rearrange("(t p) m -> p t m", p=128))
            wuk_sb = persist.tile([128, 6, HPC * DH], F16)
            nc.gpsimd.dma_start(out=wuk_sb[:],
                              in_=wuk[:, :].rearrange("(t p) m -> p t m", p=128))
            # replicated gamma/beta rows (only when LN is affine-nontrivial)
            gbt = {}
            if ln_affine:
                for nm, n in (("qg", Q_PROJ), ("qb", Q_PROJ),
                              ("kg", KV_PROJ), ("kb", KV_PROJ)):
                    t = persist.tile([128, n], F32, name=f"gb_{nm}", tag=f"gb_{nm}")
                    src = gbin[nm][:, :]
                    rep = bass.AP(tensor=src.tensor, offset=src.offset,
                                  ap=[[0, 128], src.ap[1]])
                    nc.sync.dma_start(out=t[:], in_=rep)
                    gbt[nm] = t
            qT = [persist.tile([128, L], F16, name=f"qT{i}", tag=f"qT{i}") for i in range(2)]
            kT = [persist.tile([128, L], F16, name=f"kT{i}", tag=f"kT{i}") for i in range(2)]
            cqT = [latent.tile([128, L], F16, name=f"cqT{i}", tag=f"cqT{i}") for i in range(4)]
            ckvT = [latent.tile([128, L], F16, name=f"ckvT{i}", tag=f"ckvT{i}") for i in range(6)]

            # ---------------- phase 1+2: down-proj, LN, ckv out, transposes
            with tc.tile_pool(name="wdown", bufs=1) as wdown, \
                 tc.tile_pool(name="ph1", bufs=3) as ph1, \
                 tc.tile_pool(name="ps1", bufs=2, space="PSUM") as ps1, \
                 tc.tile_pool(name="pst", bufs=2, space="PSUM") as pst:
                wdq_sb = wdown.tile([128, 8, Q_PROJ], F16)
                nc.gpsimd.dma_start(out=wdq_sb[:],
                                  in_=wdq[:, :].rearrange("(k p) c -> p k c", p=128))
                wdkv_sb = wdown.tile([128, 8, KV_PROJ], F16)
                nc.gpsimd.dma_start(out=wdkv_sb[:],
                                  in_=wdkv[:, :].rearrange("(k p) c -> p k c", p=128))
                xTr = xT[:, :].rearrange("(k p) i -> p k i", p=128)
                for it in range(IT):
                    s = it * 128
                    with nc.named_scope("down"):
                        # SWDGE cast-DMA: fp32 HBM -> fp16 SBUF
                        xt = ph1.tile([128, 8, 128], F16, tag="xt")
                        nc.gpsimd.dma_start(out=xt[:], in_=xTr[:, :, s:s + 128])
                        psq = ps1.tile([128, Q_PROJ], F32, tag="psq")
                        pskv = ps1.tile([128, KV_PROJ], F32, tag="pskv")
                        for k in range(8):
                            st, sp = (k == 0), (k == 7)
                            nc.tensor.matmul(psq[:], xt[:, k, :], wdq_sb[:, k, :],
                                             start=st, stop=sp)
                            nc.tensor.matmul(pskv[:, 0:KB1], xt[:, k, :],
                                             wdkv_sb[:, k, 0:KB1], start=st, stop=sp)
                            nc.tensor.matmul(pskv[:, KB1:KV_PROJ], xt[:, k, :],
                                             wdkv_sb[:, k, KB1:KV_PROJ],
                                             start=st, stop=sp)
                    with nc.named_scope("ln"):
                        # --- LN(q) -> fp16 latent
                        stq = ph1.tile([128, 6], F32, tag="stq")
                        nc.vector.bn_stats(stq[:], psq[:])
                        mvq = ph1.tile([128, 2], F32, tag="mvq")
                        nc.vector.bn_aggr(mvq[:], stq[:])
                        rtq = rstd_newton(ph1, mvq[:, 1:2], "q")
                        if ln_affine:
                            cqf = ph1.tile([128, Q_PROJ], F32, tag="cqf")
                            nc.vector.tensor_scalar(out=cqf[:], in0=psq[:],
                                                    scalar1=mvq[:, 0:1], scalar2=rtq[:],
                                                    op0=ALU.subtract, op1=ALU.mult)
                            nc.vector.tensor_mul(cqf[:], cqf[:], gbt["qg"][:])
                            nc.vector.tensor_add(cqf[:], cqf[:], gbt["qb"][:])
                            cqh = ph1.tile([128, Q_PROJ], F16, tag="cqh")
                            nc.vector.tensor_copy(cqh[:], cqf[:])
                        else:
                            cqh = ph1.tile([128, Q_PROJ], F16, tag="cqh")
                            nc.vector.tensor_scalar(out=cqh[:], in0=psq[:],
                                                    scalar1=mvq[:, 0:1], scalar2=rtq[:],
                                                    op0=ALU.subtract, op1=ALU.mult)
                        # --- LN(kv): fp32 output + fp16 latent
                        stkv = ph1.tile([128, 2, 6], F32, tag="stkv")
                        nc.vector.bn_stats(stkv[:, 0, :], pskv[:, 0:341])
                        nc.vector.bn_stats(stkv[:, 1, :], pskv[:, 341:682])
                        mvk = ph1.tile([128, 2], F32, tag="mvk")
                        nc.vector.bn_aggr(mvk[:], stkv[:])
                        rtk = rstd_newton(ph1, mvk[:, 1:2], "k")
                        ckv = ph1.tile([128, KV_PROJ], F32, tag="ckv")
                        nc.vector.tensor_scalar(out=ckv[:], in0=pskv[:],
                                                scalar1=mvk[:, 0:1], scalar2=rtk[:],
                                                op0=ALU.subtract, op1=ALU.mult)
                        if ln_affine:
                            nc.vector.tensor_mul(ckv[:], ckv[:], gbt["kg"][:])
                            nc.vector.tensor_add(ckv[:], ckv[:], gbt["kb"][:])
                        nc.gpsimd.dma_start(out=ckv_o[s:s + 128, :], in_=ckv[:])
                        ckvh = ph1.tile([128, KV_PAD], F16, tag="ckvh")
                        nc.vector.tensor_copy(ckvh[:, 0:KV_PROJ], ckv[:])
                        nc.vector.memset(ckvh[:, KV_PROJ:KV_PAD], 0.0)
                    with nc.named_scope("tpose"):
                        # PE transposes (fp16), evacuated by DVE/ACT copies
                        for ct in range(4):
                            pt = pst.tile([128, 128], F16, tag="pt")
                            nc.tensor.transpose(pt[:], cqh[:, ct * 128:(ct + 1) * 128],
                                                ident[:])
                            nc.any.tensor_copy(cqT[ct][:, s:s + 128], pt[:])
                        for ct in range(6):
                            pt = pst.tile([128, 128], F16, tag="pt")
                            nc.tensor.transpose(pt[:], ckvh[:, ct * 128:(ct + 1) * 128],
                                                ident[:])
                            nc.any.tensor_copy(ckvT[ct][:, s:s + 128], pt[:])

            # ---------------- phase 3: up-projections -> qT, kT (d-major, fp16)
            with tc.tile_pool(name="ps3", bufs=4, space="PSUM") as ps3, \
                 nc.named_scope("upproj"):
                for mt in range(2):
                    for ncn in range(JC):
                        t = ncn * 512
                        w = min(512, L - t)
                        pu = ps3.tile([128, 512], F32, tag="pu")
                        for ct in range(4):
                            nc.tensor.matmul(pu[:, :w],
                                             wuq_sb[:, ct, mt * 128:(mt + 1) * 128],
                                             cqT[ct][:, t:t + w],
                                             start=(ct == 0), stop=(ct == 3))
                        nc.any.tensor_copy(qT[mt][:, t:t + w], pu[:, :w])
                        pk = ps3.tile([128, 512], F32, tag="pk")
                        for ct in range(6):
                            nc.tensor.matmul(pk[:, :w],
                                             wuk_sb[:, ct, mt * 128:(mt + 1) * 128],
                                             ckvT[ct][:, t:t + w],
                                             start=(ct == 0), stop=(ct == 5))
                        nc.any.tensor_copy(kT[mt][:, t:t + w], pk[:, :w])

            # ---------------- phase 4: scores, softmax, output
            with tc.tile_pool(name="ph4", bufs=4) as ph4, \
                 tc.tile_pool(name="st4", bufs=8) as st4, \
                 tc.tile_pool(name="ps4", bufs=1, space="PSUM") as ps4, \
                 nc.named_scope("attn"):
                for mt in range(2):
                    # heads 2*mt and 2*mt+1 run paired: row groups 0-1 / 2-3
                    hA, hB = 2 * mt, 2 * mt + 1
                    for it in range(IT):
                        s = it * 128
                        psA = ps4.tile([128, L], F32, tag="psA")
                        psB = ps4.tile([128, L], F32, tag="psB")
                        for jc in range(JC):
                            t = jc * 512
                            nc.tensor.matmul(psA[:, t:t + 512],
                                             qT[mt][0:64, s:s + 128],
                                             kT[mt][0:64, t:t + 512],
                                             start=True, stop=True)
                            nc.tensor.matmul(psB[:, t:t + 512],
                                             qT[mt][64:128, s:s + 128],
                                             kT[mt][64:128, t:t + 512],
                                             start=True, stop=True)
                        for h, psx in ((hA, psA), (hB, psB)):
                            ex = ph4.tile([128, L], F32, tag="ex")
                            tot = st4.tile([128, 1], F32, tag="tot")
                            nc.scalar.activation(ex[:], psx[:], AF.Exp, scale=0.125,
                                                 accum_out=tot[:])
                            nc.vector.reciprocal(tot[:], tot[:])
                            nc.vector.tensor_scalar_mul(out=ex[:], in0=ex[:],
                                                        scalar1=tot[:])
                            nc.sync.dma_start(out=attn[h, s:s + 128, :], in_=ex[:])

    nc.compile()
    return nc


def _get_nc(L, ln_affine):
    key = (L, ln_affine)
    if key not in _NC_CACHE:
        _NC_CACHE[key] = build(L, ln_affine)
    return _NC_CACHE[key]


def _prep_in_maps(x, W_dq, W_uq, q_gamma, q_beta, W_dkv, W_ukv, kv_gamma, kv_beta,
                  ln_affine):
    f32 = lambda a: np.ascontiguousarray(np.asarray(a), dtype=np.float32)
    f16 = lambda a: np.ascontiguousarray(np.asarray(a, dtype=np.float32)).astype(np.float16)
    x = f32(x)
    xTs = [np.ascontiguousarray(x[b].T) for b in range(x.shape[0])]
    W_uq = np.asarray(W_uq, dtype=np.float32)
    W_ukv = np.asarray(W_ukv, dtype=np.float32)
    wuk_full = np.zeros((KV_PAD, D_MODEL), np.float32)
    wuk_full[:KV_PROJ] = W_ukv[:, :D_MODEL]   # K half only; V half is dead
    wdq16, wdkv16 = f16(W_dq), f16(W_dkv)
    in_maps = []
    for c in range(N_CORES):
        b, hg = c // HPC, c % HPC
        sl = slice(hg * HPC * DH, (hg + 1) * HPC * DH)
        m = {
            "xT": xTs[b],
            "wdq": wdq16,
            "wdkv": wdkv16,
            "wuq": f16(W_uq[:, sl]),
            "wuk": f16(wuk_full[:, sl]),
        }
        if ln_affine:
            m["qg"] = f32(q_gamma).reshape(1, Q_PROJ)
            m["qb"] = f32(q_beta).reshape(1, Q_PROJ)
            m["kg"] = f32(kv_gamma).reshape(1, KV_PROJ)
            m["kb"] = f32(kv_beta).reshape(1, KV_PROJ)
        in_maps.append(m)
    return x, in_maps


def kernel(x, W_dq, W_uq, q_gamma, q_beta, W_dkv, W_ukv, kv_gamma, kv_beta):
    global LAST_RESULT
    qg, qb = np.asarray(q_gamma), np.asarray(q_beta)
    kg, kb = np.asarray(kv_gamma), np.asarray(kv_beta)
    ln_affine = not (np.all(qg == 1.0) and np.all(qb == 0.0)
                     and np.all(kg == 1.0) and np.all(kb == 0.0))
    x, in_maps = _prep_in_maps(x, W_dq, W_uq, q_gamma, q_beta,
                               W_dkv, W_ukv, kv_gamma, kv_beta, ln_affine)
    Bx, L, _ = x.shape
    nc = _get_nc(L, ln_affine)
    kw = {}
    if TRACE:
        kw["trace"] = True
        if TRACE_ALL_CORES:
            kw["trace_cores"] = list(range(N_CORES))
            kw["stitch_traces"] = True
    res = run_bass_kernel_spmd(nc, in_maps, core_ids=list(range(N_CORES)), **kw)
    LAST_RESULT = res
    attn = np.empty((Bx, N_HEADS, L, L), np.float32)
    for c in range(N_CORES):
        b, hg = c // HPC, c % HPC
        attn[b, hg * HPC:(hg + 1) * HPC] = res.results[c]["attn"]
    ckv = np.stack([res.results[0]["ckv"], res.results[HPC]["ckv"]])
    return (x, attn, ckv)


# revision 12
# speedup vs baseline: 6.2113x; 1.0321x over previous
"""Multi-Head Latent Attention (MLA) prefill kernel for 8x Trainium2 NeuronCores.

Reference computes:
    compressed_q  = LN(x @ W_dq)            [B,L,512]
    Q             = compressed_q @ W_uq     [B,L,1024]
    compressed_kv = LN(x @ W_dkv)           [B,L,682]
    K             = (compressed_kv @ W_ukv)[..., :1024]
    attn          = softmax(q @ k^T / 8)    [B,16,L,L]
    returns (x, attn, compressed_kv)        (V path is dead code)

Sharding: 8 cores = 2 batches x 4 head-groups (4 heads each).  Each core
computes the full down-proj + LN for its batch (replicated within the
batch group) and the up-projections/attention for its 4 heads only.
Host-side prep (free): x is passed pre-transposed, up-proj weights are
pre-sliced per head group and cast to fp16, W_uk is zero-padded 682->768.

All matmuls run in fp16 (single-pass on the PE; fp32 is 2-pass): products
are ~0.01 scale, comfortably inside fp16 range, and the fp32 PSUM
accumulate keeps dot-product error ~1e-4 relative.  LayerNorm stats,
softmax and all outputs stay fp32.  Latent transposes use the DMA xbar
(2-byte dtype) instead of the tensor engine.
"""

import numpy as np

import concourse.bass as bass
import concourse.tile as tile
from concourse import bacc, mybir
from concourse.bass_utils import run_bass_kernel_spmd
from concourse.masks import make_identity

F32 = mybir.dt.float32
F16 = mybir.dt.float16
AF = mybir.ActivationFunctionType
ALU = mybir.AluOpType

D_MODEL = 1024
N_HEADS = 16
DH = 64
Q_PROJ = 512
KV_PROJ = 682
KV_PAD = 768          # 682 padded to 6*128 for partition tiling
LN_EPS = 1e-5
B = 2
L_FULL = 2048
HPC = 4               # heads per core
N_CORES = 8

TRACE = False
TRACE_ALL_CORES = False
LAST_RESULT = None
_NC_CACHE = {}


def build(L=L_FULL, ln_affine=False):
    IT = L // 128          # i-tiles (query rows)
    JC = max(1, L // 512)  # 512-wide column chunks
    KB1 = 512 if KV_PROJ > 512 else KV_PROJ

    nc = bacc.Bacc(None, target_bir_lowering=False)

    epsv = None

    def rstd_newton(pool, var_ap, tag):
        """rstd = 1/sqrt(var+eps) via Sqrt spline + HW reciprocal."""
        rt = pool.tile([128, 1], F32, name=f"rt_{tag}", tag=f"rt_{tag}")
        nc.scalar.activation(rt[:], var_ap, AF.Sqrt, bias=epsv[:])
        nc.vector.reciprocal(rt[:], rt[:])
        return rt

    xT = nc.dram_tensor("xT", [D_MODEL, L], F32, kind="ExternalInput")
    wdq = nc.dram_tensor("wdq", [D_MODEL, Q_PROJ], F16, kind="ExternalInput")
    wdkv = nc.dram_tensor("wdkv", [D_MODEL, KV_PROJ], F16, kind="ExternalInput")
    wuq = nc.dram_tensor("wuq", [Q_PROJ, HPC * DH], F16, kind="ExternalInput")
    wuk = nc.dram_tensor("wuk", [KV_PAD, HPC * DH], F16, kind="ExternalInput")
    attn = nc.dram_tensor("attn", [HPC, L, L], F32, kind="ExternalOutput")
    ckv_o = nc.dram_tensor("ckv", [L, KV_PROJ], F32, kind="ExternalOutput")
    gbin = {}
    if ln_affine:
        for nm, n in (("qg", Q_PROJ), ("qb", Q_PROJ),
                      ("kg", KV_PROJ), ("kb", KV_PROJ)):
            gbin[nm] = nc.dram_tensor(nm, [1, n], F32, kind="ExternalInput")

    with tile.TileContext(nc) as tc:
        with tc.tile_pool(name="persist", bufs=1) as persist, \
             tc.tile_pool(name="latent", bufs=1) as latent:
            ident = persist.tile([128, 128], F16)
            make_identity(nc, ident[:])
            epsv = persist.tile([128, 1], F32)
            nc.vector.memset(epsv[:], LN_EPS)
            wuq_sb = persist.tile([128, 4, HPC * DH], F16)
            nc.gpsimd.dma_start(out=wuq_sb[:],
                              in_=wuq[:, :].rearrange("(t p) m -> p t m", p=128))
            wuk_sb = persist.tile([128, 6, HPC * DH], F16)
            nc.gpsimd.dma_start(out=wuk_sb[:],
                              in_=wuk[:, :].rearrange("(t p) m -> p t m", p=128))
            # replicated gamma/beta rows (only when LN is affine-nontrivial)
            gbt = {}
            if ln_affine:
                for nm, n in (("qg", Q_PROJ), ("qb", Q_PROJ),
                              ("kg", KV_PROJ), ("kb", KV_PROJ)):
                    t = persist.tile([128, n], F32, name=f"gb_{nm}", tag=f"gb_{nm}")
                    src = gbin[nm][:, :]
                    rep = bass.AP(tensor=src.tensor, offset=src.offset,
                                  ap=[[0, 128], src.ap[1]])
                    nc.sync.dma_start(out=t[:], in_=rep)
                    gbt[nm] = t
            wdq_sb = persist.tile([128, 8, Q_PROJ], F16)
            nc.gpsimd.dma_start(out=wdq_sb[:],
                                in_=wdq[:, :].rearrange("(k p) c -> p k c", p=128))
            wdkv_sb = persist.tile([128, 8, KV_PROJ], F16)
            nc.gpsimd.dma_start(out=wdkv_sb[:],
                                in_=wdkv[:, :].rearrange("(k p) c -> p k c", p=128))
            qT = [persist.tile([128, L], F16, name=f"qT{i}", tag=f"qT{i}") for i in range(2)]
            kT = [persist.tile([128, L], F16, name=f"kT{i}", tag=f"kT{i}") for i in range(2)]
            cqT = [latent.tile([128, L], F16, name=f"cqT{i}", tag=f"cqT{i}") for i in range(4)]
            ckvT = [latent.tile([128, L], F16, name=f"ckvT{i}", tag=f"ckvT{i}") for i in range(6)]

            # ---------------- interleaved latent phases (A: kv, B: q) then C: attn
            with tc.tile_pool(name="a_sb", bufs=2) as a_sb, \
                 tc.tile_pool(name="b_sb", bufs=3) as b_sb, \
                 tc.tile_pool(name="a_ps", bufs=2, space="PSUM") as a_ps, \
                 tc.tile_pool(name="t_ps", bufs=1, space="PSUM") as t_ps, \
                 tc.tile_pool(name="b_ps", bufs=1, space="PSUM") as b_ps:
                xTr = xT[:, :].rearrange("(k p) i -> p k i", p=128)
                xt_all = latent.tile([128, 8, L], F16)
                nc.gpsimd.dma_start(out=xt_all[:], in_=xTr[:, :, :])

                def kv_it(it):
                    s = it * 128
                    with nc.named_scope("kv"):
                        pskv = a_ps.tile([128, KV_PROJ], F32, tag="pskv")
                        for k in range(8):
                            st, sp = (k == 0), (k == 7)
                            nc.tensor.matmul(pskv[:, 0:KB1], xt_all[:, k, s:s + 128],
                                             wdkv_sb[:, k, 0:KB1], start=st, stop=sp)
                            nc.tensor.matmul(pskv[:, KB1:KV_PROJ], xt_all[:, k, s:s + 128],
                                             wdkv_sb[:, k, KB1:KV_PROJ],
                                             start=st, stop=sp)
                        stkv = a_sb.tile([128, 2, 6], F32, tag="stkv")
                        nc.vector.bn_stats(stkv[:, 0, :], pskv[:, 0:341])
                        nc.vector.bn_stats(stkv[:, 1, :], pskv[:, 341:682])
                        mvk = a_sb.tile([128, 2], F32, tag="mvk")
                        nc.vector.bn_aggr(mvk[:], stkv[:])
                        rtk = rstd_newton(a_sb, mvk[:, 1:2], "k")
                        ckv = a_sb.tile([128, KV_PROJ], F32, tag="ckv")
                        nc.vector.tensor_scalar(out=ckv[:], in0=pskv[:],
                                                scalar1=mvk[:, 0:1], scalar2=rtk[:],
                                                op0=ALU.subtract, op1=ALU.mult)
                        if ln_affine:
                            nc.vector.tensor_mul(ckv[:], ckv[:], gbt["kg"][:])
                            nc.vector.tensor_add(ckv[:], ckv[:], gbt["kb"][:])
                        nc.gpsimd.dma_start(out=ckv_o[s:s + 128, :], in_=ckv[:])
                        ckvh = a_sb.tile([128, KV_PAD], F16, tag="ckvh")
                        nc.vector.tensor_copy(ckvh[:, 0:KV_PROJ], ckv[:])
                        nc.vector.memset(ckvh[:, KV_PROJ:KV_PAD], 0.0)
                        for ct in range(6):
                            pt = t_ps.tile([128, 128], F16, tag="aps")
                            nc.tensor.transpose(pt[:], ckvh[:, ct * 128:(ct + 1) * 128],
                                                ident[:])
                            nc.any.tensor_copy(ckvT[ct][:, s:s + 128], pt[:])

                def q_block(ig):
                    t0 = ig * 512
                    with nc.named_scope("q"):
                        for it in range(ig * 4, ig * 4 + 4):
                            s = it * 128
                            psq = b_ps.tile([128, Q_PROJ], F32, tag="psq")
                            for k in range(8):
                                nc.tensor.matmul(psq[:], xt_all[:, k, s:s + 128],
                                                 wdq_sb[:, k, :],
                                                 start=(k == 0), stop=(k == 7))
                            stq = b_sb.tile([128, 6], F32, tag="stq")
                            nc.vector.bn_stats(stq[:], psq[:])
                            mvq = b_sb.tile([128, 2], F32, tag="mvq")
                            nc.vector.bn_aggr(mvq[:], stq[:])
                            rtq = rstd_newton(b_sb, mvq[:, 1:2], "q")
                            if ln_affine:
                                cqf = b_sb.tile([128, Q_PROJ], F32, tag="cqf")
                                nc.vector.tensor_scalar(out=cqf[:], in0=psq[:],
                                                        scalar1=mvq[:, 0:1],
                                                        scalar2=rtq[:],
                                                        op0=ALU.subtract, op1=ALU.mult)
                                nc.vector.tensor_mul(cqf[:], cqf[:], gbt["qg"][:])
                                nc.vector.tensor_add(cqf[:], cqf[:], gbt["qb"][:])
                                cqh = b_sb.tile([128, Q_PROJ], F16, tag="cqh")
                                nc.vector.tensor_copy(cqh[:], cqf[:])
                            else:
                                cqh = b_sb.tile([128, Q_PROJ], F16, tag="cqh")
                                nc.vector.tensor_scalar(out=cqh[:], in0=psq[:],
                                                        scalar1=mvq[:, 0:1],
                                                        scalar2=rtq[:],
                                                        op0=ALU.subtract, op1=ALU.mult)
                            for ct in range(4):
                                pt = b_ps.tile([128, 128], F16, tag="bps")
                                nc.tensor.transpose(pt[:],
                                                    cqh[:, ct * 128:(ct + 1) * 128],
                                                    ident[:])
                                nc.any.tensor_copy(cqT[ct][:, s:s + 128], pt[:])
                        w = min(512, L - t0)
                        for mt in range(2):
                            pu = b_ps.tile([128, 512], F32, tag="bps")
                            for ct in range(4):
                                nc.tensor.matmul(pu[:, :w],
                                                 wuq_sb[:, ct, mt * 128:(mt + 1) * 128],
                                                 cqT[ct][:, t0:t0 + w],
                                                 start=(ct == 0), stop=(ct == 3))
                            nc.any.tensor_copy(qT[mt][:, t0:t0 + w], pu[:, :w])

                n_blk = max(1, IT // 4)
                for blk in range(n_blk):
                    for it in range(blk * 4, min(blk * 4 + 4, IT)):
                        kv_it(it)
                    if blk < JC:
                        q_block(blk)
                with nc.named_scope("kup"):
                    for mt in range(2):
                        for ncn in range(JC):
                            t = ncn * 512
                            w = min(512, L - t)
                            pk = t_ps.tile([128, 512], F32, tag="aps")
                            for ct in range(6):
                                nc.tensor.matmul(pk[:, :w],
                                                 wuk_sb[:, ct, mt * 128:(mt + 1) * 128],
                                                 ckvT[ct][:, t:t + w],
                                                 start=(ct == 0), stop=(ct == 5))
                            nc.any.tensor_copy(kT[mt][:, t:t + w], pk[:, :w])

            # ---------------- phase C: scores, softmax, output (paired heads)
            with tc.tile_pool(name="ph4", bufs=5) as ph4, \
                 tc.tile_pool(name="st4", bufs=16) as st4, \
                 tc.tile_pool(name="ps4", bufs=1, space="PSUM") as ps4, \
                 nc.named_scope("attn"):
                for mt in range(2):
                    hA, hB = 2 * mt, 2 * mt + 1
                    for it in range(IT):
                        s = it * 128
                        psA = ps4.tile([128, L], F32, tag="psA")
                        psB = ps4.tile([128, L], F32, tag="psB")
                        for jc in range(JC):
                            t = jc * 512
                            nc.tensor.matmul(psA[:, t:t + 512],
                                             qT[mt][0:64, s:s + 128],
                                             kT[mt][0:64, t:t + 512],
                                             start=True, stop=True)
                            nc.tensor.matmul(psB[:, t:t + 512],
                                             qT[mt][64:128, s:s + 128],
                                             kT[mt][64:128, t:t + 512],
                                             start=True, stop=True)
                        for h, psx in ((hA, psA), (hB, psB)):
                            ex = ph4.tile([128, L], F32, tag="ex")
                            tot = st4.tile([128, 1], F32, tag="tot")
                            nc.scalar.activation(ex[:], psx[:], AF.Exp, scale=0.125,
                                                 accum_out=tot[:])
                            nc.vector.reciprocal(tot[:], tot[:])
                            nc.vector.tensor_scalar_mul(out=ex[:], in0=ex[:],
                                                        scalar1=tot[:])
                            nc.sync.dma_start(out=attn[h, s:s + 128, :], in_=ex[:])

    nc.compile()
    return nc


def _get_nc(L, ln_affine):
    key = (L, ln_affine)
    if key not in _NC_CACHE:
        _NC_CACHE[key] = build(L, ln_affine)
    return _NC_CACHE[key]


def _prep_in_maps(x, W_dq, W_uq, q_gamma, q_beta, W_dkv, W_ukv, kv_gamma, kv_beta,
                  ln_affine):
    f32 = lambda a: np.ascontiguousarray(np.asarray(a), dtype=np.float32)
    f16 = lambda a: np.ascontiguousarray(np.asarray(a, dtype=np.float32)).astype(np.float16)
    x = f32(x)
    xTs = [np.ascontiguousarray(x[b].T) for b in range(x.shape[0])]
    W_uq = np.asarray(W_uq, dtype=np.float32)
    W_ukv = np.asarray(W_ukv, dtype=np.float32)
    wuk_full = np.zeros((KV_PAD, D_MODEL), np.float32)
    wuk_full[:KV_PROJ] = W_ukv[:, :D_MODEL]   # K half only; V half is dead
    wdq16, wdkv16 = f16(W_dq), f16(W_dkv)
    in_maps = []
    for c in range(N_CORES):
        b, hg = c // HPC, c % HPC
        sl = slice(hg * HPC * DH, (hg + 1) * HPC * DH)
        m = {
            "xT": xTs[b],
            "wdq": wdq16,
            "wdkv": wdkv16,
            "wuq": f16(W_uq[:, sl]),
            "wuk": f16(wuk_full[:, sl]),
        }
        if ln_affine:
            m["qg"] = f32(q_gamma).reshape(1, Q_PROJ)
            m["qb"] = f32(q_beta).reshape(1, Q_PROJ)
            m["kg"] = f32(kv_gamma).reshape(1, KV_PROJ)
            m["kb"] = f32(kv_beta).reshape(1, KV_PROJ)
        in_maps.append(m)
    return x, in_maps


def kernel(x, W_dq, W_uq, q_gamma, q_beta, W_dkv, W_ukv, kv_gamma, kv_beta):
    global LAST_RESULT
    qg, qb = np.asarray(q_gamma), np.asarray(q_beta)
    kg, kb = np.asarray(kv_gamma), np.asarray(kv_beta)
    ln_affine = not (np.all(qg == 1.0) and np.all(qb == 0.0)
                     and np.all(kg == 1.0) and np.all(kb == 0.0))
    x, in_maps = _prep_in_maps(x, W_dq, W_uq, q_gamma, q_beta,
                               W_dkv, W_ukv, kv_gamma, kv_beta, ln_affine)
    Bx, L, _ = x.shape
    nc = _get_nc(L, ln_affine)
    kw = {}
    if TRACE:
        kw["trace"] = True
        if TRACE_ALL_CORES:
            kw["trace_cores"] = list(range(N_CORES))
            kw["stitch_traces"] = True
    res = run_bass_kernel_spmd(nc, in_maps, core_ids=list(range(N_CORES)), **kw)
    LAST_RESULT = res
    attn = np.empty((Bx, N_HEADS, L, L), np.float32)
    for c in range(N_CORES):
        b, hg = c // HPC, c % HPC
        attn[b, hg * HPC:(hg + 1) * HPC] = res.results[c]["attn"]
    ckv = np.stack([res.results[0]["ckv"], res.results[HPC]["ckv"]])
    return (x, attn, ckv)
